# revision 44
# baseline (speedup 1.0000x reference)
"""AttentionGCN (GAT-style) layer on 8 trn2 NeuronCores — Bass/Tile SPMD kernel.

Math (per reference):
  A_tilde = A + I; mask = A_tilde > 0
  Wh = (H @ W.T) -> [N, HEADS, DK]
  sl[i,h] = Wh[i,h,:]@a_l[h]; sr[j,h] = Wh[j,h,:]@a_r[h]
  e[i,h,j] = leaky_relu(sl[i,h] + sr[j,h], 0.2), masked softmax over j
  out = elu(sum_j alpha[i,h,j] * Wh[j,h,:])

Key decomposition used on-device: with x = sl_i + sr_j,
  exp(lrelu(x)) = exp(x)        = v_i * u_j      for x >= 0
                = exp(0.2 x)    = v02_i * u02_j  for x < 0
so with M1 = [x>=0]*mask (a per-head tensor_scalar is_ge against the
per-partition -sr vector, then one packed-int32 bitwise AND with the mask —
masks are exact {0,1.0} bf16 so the AND equals the product) and
M0 = mask - M1, the softmax numerator aggregation becomes plain matmuls:
  Num[d,i] = v_i * sum_j (Wh*u)[j,d] M1[j,i] + v02_i * sum_j (Wh*u02)[j,d] M0[j,i]
  Z[i]     = same with ones column appended to the scaled-Wh matrices.
M0's aggregation is realized by accumulating (-Wh*u02)@M1 + (Wh*u02)@mask
into one PSUM bank per head.

Sharding: query rows i split 512/core across 8 cores; mask shards are
pre-transposed/binarized on host; everything else replicated. No collectives.
"""
import os
import sys

import numpy as np

if "/opt/trn_rl_repo" not in sys.path:
    sys.path.insert(0, "/opt/trn_rl_repo")

import ml_dtypes
from contextlib import ExitStack

import concourse.bass as bass
import concourse.tile as tile
from concourse import bacc, mybir
from concourse.bass_utils import run_bass_kernel_spmd

N, IN_DIM, OUT_DIM, HEADS, DK = 4096, 256, 256, 4, 64
NCORES = 8
SH = N // NCORES            # 512 query rows per core
P = 128                     # partitions
JC = N // P                 # 32 j-chunks
ICN = SH // P               # 4 i-chunks
GRP = 4                     # j-chunks per prep group
NG = JC // GRP              # 4 groups
W8 = IN_DIM + 8             # wrhs cols: 256 W.T + 4 wl + 4 wr
D1 = DK + 1

F32 = mybir.dt.float32
F32R = mybir.dt.float32r
BF16 = mybir.dt.bfloat16
AF = mybir.ActivationFunctionType
ALU = mybir.AluOpType

# tuned constants (via on-device A/B benchmarking)
KN_WARM = 0
KN_XOR = 0
KN_M1B = 4
KN_FILL = 0
KN_SPLIT = 0
KN_PREM = 0


def build():
    nc = bacc.Bacc("TRN2", target_bir_lowering=False, debug=False)
    mT_e = nc.declare_dram_parameter("mT", [N, SH], BF16, isOutput=False)
    hT_e = nc.declare_dram_parameter("hT", [IN_DIM, N], BF16, isOutput=False)
    hTo_e = nc.declare_dram_parameter("hTo", [IN_DIM, SH], F32R, isOutput=False)
    wrhs_e = nc.declare_dram_parameter("wrhs", [IN_DIM, W8], F32R, isOutput=False)
    id_e = nc.declare_dram_parameter("ident", [P, P], F32, isOutput=False)
    out_e = nc.declare_dram_parameter("out", [SH, OUT_DIM], F32, isOutput=True)

    with tile.TileContext(nc) as tc, ExitStack() as ctx:
        consts = ctx.enter_context(tc.tile_pool(name="consts", bufs=1))
        mpool = ctx.enter_context(tc.tile_pool(name="mask", bufs=1))
        hpool = ctx.enter_context(tc.tile_pool(name="hT", bufs=16))
        whpool = ctx.enter_context(tc.tile_pool(name="whsb", bufs=2))
        lupool = ctx.enter_context(tc.tile_pool(name="lu", bufs=NG))
        spool = ctx.enter_context(tc.tile_pool(name="small", bufs=NG))
        bcpool = ctx.enter_context(tc.tile_pool(name="bc", bufs=1))
        m1pool = ctx.enter_context(tc.tile_pool(name="m1", bufs=KN_M1B))
        finpool = ctx.enter_context(tc.tile_pool(name="fin", bufs=4))
        opool = ctx.enter_context(tc.tile_pool(name="o", bufs=2))

        whps = ctx.enter_context(tc.tile_pool(name="whps", bufs=2, space="PSUM"))
        sps = ctx.enter_context(tc.tile_pool(name="sps", bufs=1, space="PSUM"))
        gps = ctx.enter_context(tc.tile_pool(name="gps", bufs=1, space="PSUM"))
        tps = ctx.enter_context(tc.tile_pool(name="tps", bufs=1, space="PSUM"))

        # ---------- static inputs ----------
        wrhs = consts.tile([P, 2, W8], F32R)     # two K-halves of [W.T|wl|wr]
        hTo = consts.tile([P, 2, SH], F32R)
        ident = consts.tile([P, P], F32)
        ones1 = consts.tile([1, P], F32)
        nc.sync.dma_start(wrhs[:, 0, :], wrhs_e[0:P, :])
        nc.sync.dma_start(wrhs[:, 1, :], wrhs_e[P : 2 * P, :])
        nc.sync.dma_start(hTo[:, 0, :], hTo_e[0:P, :])
        nc.sync.dma_start(hTo[:, 1, :], hTo_e[P : 2 * P, :])
        nc.sync.dma_start(ident[:], id_e[:])
        nc.vector.memset(ones1[:], 1.0)
        wrhsb = consts.tile([P, 2, W8], BF16)
        nc.vector.tensor_copy(wrhsb[:], wrhs[:])

        if KN_WARM or KN_FILL:
            wz = consts.tile([1, SH], BF16)
            wo = consts.tile([1, P], BF16)
            nc.vector.memset(wz[:], 0.0)
            nc.vector.memset(wo[:], 1.0)
            if KN_WARM:
                wps = tps.tile([P, SH], F32, tag="tp", name="warm")
                for r in range(KN_WARM):
                    nc.tensor.matmul(wps[:], wo[:], wz[:], start=(r == 0),
                                     stop=(r == KN_WARM - 1))

        # ---------- score_l row vector + bcast tiles ----------
        with nc.named_scope("prep_scores"):
            slps = sps.tile([8, SH], F32, tag="s")
            nc.tensor.matmul(slps[:], wrhs[:, 0, IN_DIM:], hTo[:, 0, :],
                             start=True, stop=False)
            nc.tensor.matmul(slps[:], wrhs[:, 1, IN_DIM:], hTo[:, 1, :],
                             start=False, stop=True)
            slT = consts.tile([8, SH], F32)
            nc.vector.tensor_copy(slT[:], slps[:])
            # move each head's score_l row to partition 0 (matmul rhs needs base 0)
            slF = consts.tile([1, HEADS, SH], F32)
            for h in range(HEADS):
                nc.sync.dma_start(slF[0:1, h, :], slT[h : h + 1, :])

            # own-row scores in [i, type] layout for the finalize scalings
            vps = sps.tile([P, 8], F32, tag="s", name="vps")
            sco = consts.tile([P, ICN, 8], F32)
            for icx in range(ICN):
                nc.tensor.matmul(vps[:], hTo[:, 0, icx * P : (icx + 1) * P],
                                 wrhs[:, 0, IN_DIM:], start=True, stop=False)
                nc.tensor.matmul(vps[:], hTo[:, 1, icx * P : (icx + 1) * P],
                                 wrhs[:, 1, IN_DIM:], start=False, stop=True)
                nc.vector.tensor_copy(sco[:, icx, :], vps[:])
            v_all = consts.tile([P, ICN, HEADS], F32)
            v02_all = consts.tile([P, ICN, HEADS], F32)
            nc.scalar.activation(v_all[:], sco[:, :, 0:4], AF.Exp)
            nc.scalar.activation(v02_all[:], sco[:, :, 0:4], AF.Exp, scale=0.2)

            # broadcast score_l along partitions: bc16[j, h, i] = sl[i, h]
            bc16 = bcpool.tile([P, HEADS, SH], BF16)
            for h in range(HEADS):
                bps = sps.tile([P, SH], F32, tag="s", name="bps")
                nc.tensor.matmul(bps[:], ones1[:], slF[0:1, h, :],
                                 start=True, stop=True)
                nc.vector.tensor_copy(bc16[:, h, :], bps[:])

        # ---------- per-group: Wh matmuls, scores, scaled lhsT builds ----------
        lu_g, lun_g, lup_g, nsr_g = [], [], [], []
        deferred = []
        for g in range(NG):
            with nc.named_scope(f"prep_g{g}"):
                whsb = whpool.tile([P, GRP, W8], F32, tag="whsb")
                for cc in range(GRP):
                    c = g * GRP + cc
                    ht0 = hpool.tile([P, P], BF16, tag="ht0")
                    ht1 = hpool.tile([P, P], BF16, tag="ht1")
                    nc.sync.dma_start(ht0[:], hT_e[0:P, c * P : (c + 1) * P])
                    nc.sync.dma_start(ht1[:], hT_e[P : 2 * P, c * P : (c + 1) * P])
                    whp = whps.tile([P, W8], F32)
                    nc.tensor.matmul(whp[:], ht0[:], wrhsb[:, 0, :],
                                     start=True, stop=False)
                    nc.tensor.matmul(whp[:], ht1[:], wrhsb[:, 1, :],
                                     start=False, stop=True)
                    nc.scalar.copy(whsb[:, cc, :], whp[:])
                    for _f in range(KN_FILL):
                        fps = tps.tile([P, P], F32, tag="tp", name="fill")
                        nc.tensor.matmul(fps[:], wo[:], wz[0:1, 0:P],
                                         start=True, stop=True)

                # u = exp(sr), u02 = exp(0.2 sr), nsr = -sr  (sr = cols 260:264)
                u = spool.tile([P, GRP, HEADS], F32, tag="u")
                u02 = spool.tile([P, GRP, HEADS], F32, tag="u02")
                nsr = spool.tile([P, GRP, HEADS], F32, tag="nsr")
                nc.scalar.activation(u[:], whsb[:, :, IN_DIM + 4 :], AF.Exp)
                nc.scalar.activation(u02[:], whsb[:, :, IN_DIM + 4 :], AF.Exp,
                                     scale=0.2)
                nc.vector.tensor_scalar_mul(nsr[:], whsb[:, :, IN_DIM + 4 :], -1.0)

                # lhsT builds: lu = [Wh*u | u], lup = [Wh*u02 | u02], lun = -lup
                lu = lupool.tile([P, GRP, HEADS, D1], BF16, tag="lu")
                lup = lupool.tile([P, GRP, HEADS, D1], BF16, tag="lup")
                lun = lupool.tile([P, GRP, HEADS, D1], BF16, tag="lun")
                wh4 = whsb[:, :, 0:IN_DIM].rearrange("p g (h d) -> p g h d", h=HEADS)
                ub = u[:].broadcast_to([P, GRP, HEADS, DK])
                u02b = u02[:].broadcast_to([P, GRP, HEADS, DK])
                HB = 2 if KN_SPLIT else HEADS
                nc.vector.tensor_tensor(lu[:, :, 0:HB, 0:DK], wh4[:, :, 0:HB, :],
                                        ub[:, :, 0:HB, :], ALU.mult)
                nc.vector.tensor_copy(lu[:, :, 0:HB, DK], u[:, :, 0:HB])
                nc.vector.tensor_tensor(lup[:, :, 0:HB, 0:DK], wh4[:, :, 0:HB, :],
                                        u02b[:, :, 0:HB, :], ALU.mult)
                nc.vector.tensor_copy(lup[:, :, 0:HB, DK], u02[:, :, 0:HB])
                nc.vector.tensor_scalar_mul(lun[:, :, 0:HB, :], lup[:, :, 0:HB, :],
                                            -1.0)
                if KN_SPLIT:
                    deferred.append((lu, lup, lun, wh4, ub, u02b, u, u02))
                lu_g.append(lu); lun_g.append(lun); lup_g.append(lup)
                nsr_g.append(nsr)

        m_tiles = []
        for c in range(JC):
            mt = mpool.tile([P, 1, SH], BF16, tag=f"m{c}", name=f"m{c}")
            nc.sync.dma_start(mt[:, 0, :], mT_e[c * P : (c + 1) * P, :])
            m_tiles.append(mt[:])

        # ---------- main loops: two phases of two heads ----------
        oT = [opool.tile([P, OUT_DIM], F32, tag=f"oT{icx}", name=f"oT{icx}")
              for icx in range(ICN)]
        for ph in range(2):
            h0 = 2 * ph
            with nc.named_scope(f"phase{ph}"):
                g1 = [gps.tile([D1, SH], F32, tag=f"g1_{hh}", name=f"g1_{hh}")
                      for hh in range(2)]
                g23 = [gps.tile([D1, SH], F32, tag=f"g23_{hh}", name=f"g23_{hh}")
                       for hh in range(2)]
                if KN_PREM:
                    # mask-only aggregation pre-block: no DVE dependency, runs
                    # dense at phase start while the M1 pipeline fills
                    for hh in range(2):
                        h = h0 + hh
                        for c in range(JC):
                            g, cc = divmod(c, GRP)
                            nc.tensor.matmul(g23[hh][:], lup_g[g][:, cc, h, :],
                                             m_tiles[c][:, 0, :],
                                             start=(c == 0), stop=False)
                for c in range(JC):
                    g, cc = divmod(c, GRP)
                    c2 = m1pool.tile([P, 2, SH], BF16, tag="c2")
                    m14 = m1pool.tile([P, 2, SH], BF16, tag="m14")
                    for hh in range(2):
                        nc.vector.tensor_scalar(
                            c2[:, hh, :], bc16[:, h0 + hh, :],
                            nsr_g[g][:, cc, h0 + hh : h0 + hh + 1], None,
                            op0=ALU.is_ge)
                    # masks are exact {0,1.0} bf16: M1 = c & m on packed int32
                    mi = m_tiles[c].bitcast(mybir.dt.uint32)
                    mbi = mi.broadcast_to([P, 2, SH // 2])
                    nc.vector.tensor_tensor(m14[:].bitcast(mybir.dt.uint32),
                                            c2[:].bitcast(mybir.dt.uint32), mbi,
                                            ALU.bitwise_and)
                    xor_chunk = (c % 5) < KN_XOR
                    if xor_chunk:
                        m0 = m1pool.tile([P, 2, SH], BF16, tag="m0")
                        nc.vector.tensor_tensor(m0[:].bitcast(mybir.dt.uint32),
                                                m14[:].bitcast(mybir.dt.uint32),
                                                mbi, ALU.bitwise_xor)
                    nc.tensor.matmul(g1[0][:], lu_g[g][:, cc, h0, :],
                                     m14[:, 0, :],
                                     start=(c == 0), stop=(c == JC - 1))
                    nc.tensor.matmul(g1[1][:], lu_g[g][:, cc, h0 + 1, :],
                                     m14[:, 1, :],
                                     start=(c == 0), stop=(c == JC - 1))
                    for hh in range(2):
                        h = h0 + hh
                        if KN_PREM:
                            nc.tensor.matmul(g23[hh][:], lun_g[g][:, cc, h, :],
                                             m14[:, hh, :],
                                             start=False, stop=(c == JC - 1))
                        elif xor_chunk:
                            nc.tensor.matmul(g23[hh][:], lup_g[g][:, cc, h, :],
                                             m0[:, hh, :],
                                             start=(c == 0), stop=(c == JC - 1))
                        else:
                            nc.tensor.matmul(g23[hh][:], lun_g[g][:, cc, h, :],
                                             m14[:, hh, :],
                                             start=(c == 0), stop=False)
                            nc.tensor.matmul(g23[hh][:], lup_g[g][:, cc, h, :],
                                             m_tiles[c][:, 0, :],
                                             start=False, stop=(c == JC - 1))

                if ph == 0 and KN_SPLIT:
                    for (lu, lup, lun, wh4, ub, u02b, u, u02) in deferred:
                        nc.vector.tensor_tensor(lu[:, :, 2:4, 0:DK],
                                                wh4[:, :, 2:4, :],
                                                ub[:, :, 2:4, :], ALU.mult)
                        nc.vector.tensor_copy(lu[:, :, 2:4, DK], u[:, :, 2:4])
                        nc.vector.tensor_tensor(lup[:, :, 2:4, 0:DK],
                                                wh4[:, :, 2:4, :],
                                                u02b[:, :, 2:4, :], ALU.mult)
                        nc.vector.tensor_copy(lup[:, :, 2:4, DK], u02[:, :, 2:4])
                        nc.vector.tensor_scalar_mul(lun[:, :, 2:4, :],
                                                    lup[:, :, 2:4, :], -1.0)

                # ---------- finalize heads: transpose, combine, divide ----------
                for hh in range(2):
                    h = h0 + hh
                    with nc.named_scope(f"fin{h}"):
                        g1sb = finpool.tile([D1, SH], F32, tag="g1sb")
                        g23sb = finpool.tile([D1, SH], F32, tag="g23sb")
                        nc.scalar.copy(g1sb[:], g1[hh][:])
                        nc.scalar.copy(g23sb[:], g23[hh][:])
                        for icx in range(ICN):
                            tp = tps.tile([P, 2 * D1], F32, tag="tp", name="tp")
                            isl = slice(icx * P, (icx + 1) * P)
                            nc.tensor.transpose(tp[:, 0:D1], g1sb[:, isl],
                                                ident[:D1, :D1])
                            nc.tensor.transpose(tp[:, D1 : 2 * D1], g23sb[:, isl],
                                                ident[:D1, :D1])
                            q1 = finpool.tile([P, D1], F32, tag="q1")
                            q2 = finpool.tile([P, D1], F32, tag="q2")
                            nc.vector.tensor_scalar_mul(
                                q1[:], tp[:, 0:D1], v_all[:, icx, h : h + 1])
                            nc.vector.scalar_tensor_tensor(
                                q2[:], tp[:, D1 : 2 * D1],
                                v02_all[:, icx, h : h + 1], q1[:],
                                op0=ALU.mult, op1=ALU.add)
                            rz = finpool.tile([P, 1], F32, tag="rz")
                            nc.vector.reciprocal(rz[:], q2[:, DK : DK + 1])
                            nc.vector.tensor_scalar_mul(
                                oT[icx][:, h * DK : (h + 1) * DK],
                                q2[:, 0:DK], rz[:])

        # ---------- elu + output ----------
        with nc.named_scope("elu_out"):
            for icx in range(ICN):
                xm = finpool.tile([P, OUT_DIM], F32, tag="xm")
                ge = finpool.tile([P, OUT_DIM], F32, tag="ge")
                r2 = finpool.tile([P, OUT_DIM], F32, tag="r2")
                nc.vector.tensor_scalar_min(xm[:], oT[icx][:], 0.0)
                nc.scalar.activation(ge[:], xm[:], AF.Exp)
                nc.vector.scalar_tensor_tensor(r2[:], oT[icx][:], 0.0, ge[:],
                                               op0=ALU.max, op1=ALU.add)
                nc.vector.tensor_scalar_add(r2[:], r2[:], -1.0)
                nc.sync.dma_start(out_e[icx * P : (icx + 1) * P, :], r2[:])

    nc.compile()
    return nc


_NC_CACHE = None


def _get_nc():
    global _NC_CACHE
    if _NC_CACHE is None:
        _NC_CACHE = build()
    return _NC_CACHE


def make_in_maps(H, A, W, a_l, a_r):
    H = np.asarray(H, dtype=np.float32)
    A = np.asarray(A, dtype=np.float32)
    W = np.asarray(W, dtype=np.float32)
    a_l = np.asarray(a_l, dtype=np.float32)
    a_r = np.asarray(a_r, dtype=np.float32)

    mask = (A > 0).astype(np.float32)
    np.fill_diagonal(mask, 1.0)  # A_tilde = A + I > 0

    hT = np.ascontiguousarray(H.T).astype(ml_dtypes.bfloat16)
    # wl[k,h] = sum_d W[h*DK+d, k] * a_l[h, d]
    W4 = W.reshape(HEADS, DK, IN_DIM)
    wl = np.einsum("hdk,hd->kh", W4, a_l).astype(np.float32)
    wr = np.einsum("hdk,hd->kh", W4, a_r).astype(np.float32)
    wrhs = np.ascontiguousarray(np.concatenate([W.T, wl, wr], axis=1))
    ident = np.eye(P, dtype=np.float32)

    in_maps = []
    for c in range(NCORES):
        rows = slice(c * SH, (c + 1) * SH)
        mT = np.ascontiguousarray(mask[rows, :].T).astype(ml_dtypes.bfloat16)
        hTo = np.ascontiguousarray(H[rows, :].T)
        in_maps.append({"mT": mT, "hT": hT, "hTo": hTo, "wrhs": wrhs,
                        "ident": ident})
    return in_maps


def run(H, A, W, a_l, a_r, trace=False):
    nc = _get_nc()
    in_maps = make_in_maps(H, A, W, a_l, a_r)
    res = run_bass_kernel_spmd(nc, in_maps, core_ids=list(range(NCORES)),
                               trace=trace)
    out = np.concatenate([res.results[c]["out"] for c in range(NCORES)], axis=0)
    return out, res


def kernel(H, A, W, a_l, a_r):
    out, _ = run(H, A, W, a_l, a_r, trace=False)
    return out


# revision 45
# speedup vs baseline: 1.2101x; 1.2101x over previous
"""AttentionGCN (GAT-style) layer on 8 trn2 NeuronCores — Bass/Tile SPMD kernel.

Math (per reference):
  A_tilde = A + I; mask = A_tilde > 0
  Wh = (H @ W.T) -> [N, HEADS, DK]
  sl[i,h] = Wh[i,h,:]@a_l[h]; sr[j,h] = Wh[j,h,:]@a_r[h]
  e[i,h,j] = leaky_relu(sl[i,h] + sr[j,h], 0.2), masked softmax over j
  out = elu(sum_j alpha[i,h,j] * Wh[j,h,:])

Key decomposition used on-device: with x = sl_i + sr_j,
  exp(lrelu(x)) = exp(x)        = v_i * u_j      for x >= 0
                = exp(0.2 x)    = v02_i * u02_j  for x < 0
so with M1 = [x>=0]*mask (a per-head tensor_scalar is_ge against the
per-partition -sr vector, then one packed-int32 bitwise AND with the mask —
masks are exact {0,1.0} bf16 so the AND equals the product) and
M0 = mask - M1, the softmax numerator aggregation becomes plain matmuls:
  Num[d,i] = v_i * sum_j (Wh*u)[j,d] M1[j,i] + v02_i * sum_j (Wh*u02)[j,d] M0[j,i]
  Z[i]     = same with ones column appended to the scaled-Wh matrices.
M0's aggregation is realized by accumulating (-Wh*u02)@M1 + (Wh*u02)@mask
into one PSUM bank per head.

Sharding: query rows i split 512/core across 8 cores; mask shards are
pre-transposed/binarized on host; everything else replicated. No collectives.
"""
import os
import sys

import numpy as np

if "/opt/trn_rl_repo" not in sys.path:
    sys.path.insert(0, "/opt/trn_rl_repo")

import ml_dtypes
from contextlib import ExitStack

import concourse.bass as bass
import concourse.tile as tile
from concourse import bacc, mybir
from concourse.bass_utils import run_bass_kernel_spmd

N, IN_DIM, OUT_DIM, HEADS, DK = 4096, 256, 256, 4, 64
NCORES = 8
SH = N // NCORES            # 512 query rows per core
P = 128                     # partitions
JC = N // P                 # 32 j-chunks
ICN = SH // P               # 4 i-chunks
GRP = 4                     # j-chunks per prep group
NG = JC // GRP              # 4 groups
W8 = IN_DIM + 8             # wrhs cols: 256 W.T + 4 wl + 4 wr
D1 = DK + 1

F32 = mybir.dt.float32
F32R = mybir.dt.float32r
BF16 = mybir.dt.bfloat16
AF = mybir.ActivationFunctionType
ALU = mybir.AluOpType

# tuned constants (via on-device A/B benchmarking)
KN_WARM = 0
KN_XOR = 0
KN_M1B = 4
KN_FILL = 0
KN_SPLIT = 0
KN_PREM = 0


def build():
    nc = bacc.Bacc("TRN2", target_bir_lowering=False, debug=False)
    mT_e = nc.declare_dram_parameter("mT", [N, SH], BF16, isOutput=False)
    hT_e = nc.declare_dram_parameter("hT", [IN_DIM, N], BF16, isOutput=False)
    hTo_e = nc.declare_dram_parameter("hTo", [IN_DIM, SH], F32R, isOutput=False)
    wrhs_e = nc.declare_dram_parameter("wrhs", [IN_DIM, W8], F32R, isOutput=False)
    id_e = nc.declare_dram_parameter("ident", [P, P], F32, isOutput=False)
    out_e = nc.declare_dram_parameter("out", [SH, OUT_DIM], F32, isOutput=True)

    with tile.TileContext(nc) as tc, ExitStack() as ctx:
        consts = ctx.enter_context(tc.tile_pool(name="consts", bufs=1))
        mpool = ctx.enter_context(tc.tile_pool(name="mask", bufs=1))
        hpool = ctx.enter_context(tc.tile_pool(name="hT", bufs=16))
        whpool = ctx.enter_context(tc.tile_pool(name="whsb", bufs=2))
        lupool = ctx.enter_context(tc.tile_pool(name="lu", bufs=NG))
        spool = ctx.enter_context(tc.tile_pool(name="small", bufs=NG))
        bcpool = ctx.enter_context(tc.tile_pool(name="bc", bufs=1))
        m1pool = ctx.enter_context(tc.tile_pool(name="m1", bufs=KN_M1B))
        finpool = ctx.enter_context(tc.tile_pool(name="fin", bufs=4))
        opool = ctx.enter_context(tc.tile_pool(name="o", bufs=2))

        whps = ctx.enter_context(tc.tile_pool(name="whps", bufs=2, space="PSUM"))
        sps = ctx.enter_context(tc.tile_pool(name="sps", bufs=1, space="PSUM"))
        gps = ctx.enter_context(tc.tile_pool(name="gps", bufs=1, space="PSUM"))
        tps = ctx.enter_context(tc.tile_pool(name="tps", bufs=1, space="PSUM"))

        # ---------- static inputs ----------
        wrhs = consts.tile([P, 2, W8], F32R)     # two K-halves of [W.T|wl|wr]
        hTo = consts.tile([P, 2, SH], F32R)
        ident = consts.tile([P, P], F32)
        ones1 = consts.tile([1, P], F32)
        nc.sync.dma_start(wrhs[:, 0, :], wrhs_e[0:P, :])
        nc.sync.dma_start(wrhs[:, 1, :], wrhs_e[P : 2 * P, :])
        nc.sync.dma_start(hTo[:, 0, :], hTo_e[0:P, :])
        nc.sync.dma_start(hTo[:, 1, :], hTo_e[P : 2 * P, :])
        nc.sync.dma_start(ident[:], id_e[:])
        nc.vector.memset(ones1[:], 1.0)
        wrhsb = consts.tile([P, 2, W8], BF16)
        nc.vector.tensor_copy(wrhsb[:], wrhs[:])

        if KN_WARM or KN_FILL:
            wz = consts.tile([1, SH], BF16)
            wo = consts.tile([1, P], BF16)
            nc.vector.memset(wz[:], 0.0)
            nc.vector.memset(wo[:], 1.0)
            if KN_WARM:
                wps = tps.tile([P, SH], F32, tag="tp", name="warm")
                for r in range(KN_WARM):
                    nc.tensor.matmul(wps[:], wo[:], wz[:], start=(r == 0),
                                     stop=(r == KN_WARM - 1))

        # ---------- score_l row vector + bcast tiles ----------
        with nc.named_scope("prep_scores"):
            slps = sps.tile([8, SH], F32, tag="s")
            nc.tensor.matmul(slps[:], wrhs[:, 0, IN_DIM:], hTo[:, 0, :],
                             start=True, stop=False)
            nc.tensor.matmul(slps[:], wrhs[:, 1, IN_DIM:], hTo[:, 1, :],
                             start=False, stop=True)
            slT = consts.tile([8, SH], F32)
            nc.vector.tensor_copy(slT[:], slps[:])
            # move each head's score_l row to partition 0 (matmul rhs needs base 0)
            slF = consts.tile([1, HEADS, SH], F32)
            for h in range(HEADS):
                nc.sync.dma_start(slF[0:1, h, :], slT[h : h + 1, :])

            # own-row scores in [i, type] layout for the finalize scalings
            vps = sps.tile([P, 8], F32, tag="s", name="vps")
            sco = consts.tile([P, ICN, 8], F32)
            for icx in range(ICN):
                nc.tensor.matmul(vps[:], hTo[:, 0, icx * P : (icx + 1) * P],
                                 wrhs[:, 0, IN_DIM:], start=True, stop=False)
                nc.tensor.matmul(vps[:], hTo[:, 1, icx * P : (icx + 1) * P],
                                 wrhs[:, 1, IN_DIM:], start=False, stop=True)
                nc.vector.tensor_copy(sco[:, icx, :], vps[:])
            v_all = consts.tile([P, ICN, HEADS], F32)
            v02_all = consts.tile([P, ICN, HEADS], F32)
            nc.scalar.activation(v_all[:], sco[:, :, 0:4], AF.Exp)
            nc.scalar.activation(v02_all[:], sco[:, :, 0:4], AF.Exp, scale=0.2)

            # broadcast score_l along partitions: bc16[j, h, i] = sl[i, h]
            bc16 = bcpool.tile([P, HEADS, SH], BF16)
            for h in range(HEADS):
                bps = sps.tile([P, SH], F32, tag="s", name="bps")
                nc.tensor.matmul(bps[:], ones1[:], slF[0:1, h, :],
                                 start=True, stop=True)
                nc.vector.tensor_copy(bc16[:, h, :], bps[:])

        # ---------- per-group: Wh matmuls, scores, scaled lhsT builds ----------
        lu_g, lun_g, lup_g, nsr_g = [], [], [], []
        deferred = []
        for g in range(NG):
            with nc.named_scope(f"prep_g{g}"):
                whsb = whpool.tile([P, GRP, W8], F32, tag="whsb")
                for cc in range(GRP):
                    c = g * GRP + cc
                    ht0 = hpool.tile([P, P], BF16, tag="ht0")
                    ht1 = hpool.tile([P, P], BF16, tag="ht1")
                    nc.sync.dma_start(ht0[:], hT_e[0:P, c * P : (c + 1) * P])
                    nc.sync.dma_start(ht1[:], hT_e[P : 2 * P, c * P : (c + 1) * P])
                    whp = whps.tile([P, W8], F32)
                    nc.tensor.matmul(whp[:], ht0[:], wrhsb[:, 0, :],
                                     start=True, stop=False)
                    nc.tensor.matmul(whp[:], ht1[:], wrhsb[:, 1, :],
                                     start=False, stop=True)
                    nc.scalar.copy(whsb[:, cc, :], whp[:])
                    for _f in range(KN_FILL):
                        fps = tps.tile([P, P], F32, tag="tp", name="fill")
                        nc.tensor.matmul(fps[:], wo[:], wz[0:1, 0:P],
                                         start=True, stop=True)

                # u = exp(sr), u02 = exp(0.2 sr), nsr = -sr  (sr = cols 260:264)
                u = spool.tile([P, GRP, HEADS], F32, tag="u")
                u02 = spool.tile([P, GRP, HEADS], F32, tag="u02")
                nsr = spool.tile([P, GRP, HEADS], F32, tag="nsr")
                nc.scalar.activation(u[:], whsb[:, :, IN_DIM + 4 :], AF.Exp)
                nc.scalar.activation(u02[:], whsb[:, :, IN_DIM + 4 :], AF.Exp,
                                     scale=0.2)
                nc.vector.tensor_scalar_mul(nsr[:], whsb[:, :, IN_DIM + 4 :], -1.0)

                # lhsT builds: lu = [Wh*u | u], lup = [Wh*u02 | u02], lun = -lup
                lu = lupool.tile([P, GRP, HEADS, D1], BF16, tag="lu")
                lup = lupool.tile([P, GRP, HEADS, D1], BF16, tag="lup")
                lun = lupool.tile([P, GRP, HEADS, D1], BF16, tag="lun")
                wh4 = whsb[:, :, 0:IN_DIM].rearrange("p g (h d) -> p g h d", h=HEADS)
                ub = u[:].broadcast_to([P, GRP, HEADS, DK])
                u02b = u02[:].broadcast_to([P, GRP, HEADS, DK])
                HB = 2 if KN_SPLIT else HEADS
                nc.vector.tensor_tensor(lu[:, :, 0:HB, 0:DK], wh4[:, :, 0:HB, :],
                                        ub[:, :, 0:HB, :], ALU.mult)
                nc.vector.tensor_copy(lu[:, :, 0:HB, DK], u[:, :, 0:HB])
                nc.vector.tensor_tensor(lup[:, :, 0:HB, 0:DK], wh4[:, :, 0:HB, :],
                                        u02b[:, :, 0:HB, :], ALU.mult)
                nc.vector.tensor_copy(lup[:, :, 0:HB, DK], u02[:, :, 0:HB])
                nc.vector.tensor_scalar_mul(lun[:, :, 0:HB, :], lup[:, :, 0:HB, :],
                                            -1.0)
                if KN_SPLIT:
                    deferred.append((lu, lup, lun, wh4, ub, u02b, u, u02))
                lu_g.append(lu); lun_g.append(lun); lup_g.append(lup)
                nsr_g.append(nsr)

        m_tiles = []
        for c in range(JC):
            mt = mpool.tile([P, 1, SH], BF16, tag=f"m{c}", name=f"m{c}")
            nc.sync.dma_start(mt[:, 0, :], mT_e[c * P : (c + 1) * P, :])
            m_tiles.append(mt[:])

        # ---------- main loops: two phases of two heads ----------
        oT = [opool.tile([P, OUT_DIM], F32, tag=f"oT{icx}", name=f"oT{icx}")
              for icx in range(ICN)]
        for ph in range(2):
            h0 = 2 * ph
            with nc.named_scope(f"phase{ph}"):
                g1 = [gps.tile([D1, SH], F32, tag=f"g1_{hh}", name=f"g1_{hh}")
                      for hh in range(2)]
                g23 = [gps.tile([D1, SH], F32, tag=f"g23_{hh}", name=f"g23_{hh}")
                       for hh in range(2)]
                if KN_PREM:
                    # mask-only aggregation pre-block: no DVE dependency, runs
                    # dense at phase start while the M1 pipeline fills
                    for hh in range(2):
                        h = h0 + hh
                        for c in range(JC):
                            g, cc = divmod(c, GRP)
                            nc.tensor.matmul(g23[hh][:], lup_g[g][:, cc, h, :],
                                             m_tiles[c][:, 0, :],
                                             start=(c == 0), stop=False)
                for c in range(JC):
                    g, cc = divmod(c, GRP)
                    c2 = m1pool.tile([P, 2, SH], BF16, tag="c2")
                    m14 = m1pool.tile([P, 2, SH], BF16, tag="m14")
                    for hh in range(2):
                        nc.vector.tensor_scalar(
                            c2[:, hh, :], bc16[:, h0 + hh, :],
                            nsr_g[g][:, cc, h0 + hh : h0 + hh + 1], None,
                            op0=ALU.is_ge)
                    # masks are exact {0,1.0} bf16: M1 = c & m on packed int32
                    mi = m_tiles[c].bitcast(mybir.dt.uint32)
                    mbi = mi.broadcast_to([P, 2, SH // 2])
                    nc.vector.tensor_tensor(m14[:].bitcast(mybir.dt.uint32),
                                            c2[:].bitcast(mybir.dt.uint32), mbi,
                                            ALU.bitwise_and)
                    xor_chunk = (c % 5) < KN_XOR
                    if xor_chunk:
                        m0 = m1pool.tile([P, 2, SH], BF16, tag="m0")
                        nc.vector.tensor_tensor(m0[:].bitcast(mybir.dt.uint32),
                                                m14[:].bitcast(mybir.dt.uint32),
                                                mbi, ALU.bitwise_xor)
                    nc.tensor.matmul(g1[0][:], lu_g[g][:, cc, h0, :],
                                     m14[:, 0, :],
                                     start=(c == 0), stop=(c == JC - 1))
                    nc.tensor.matmul(g1[1][:], lu_g[g][:, cc, h0 + 1, :],
                                     m14[:, 1, :],
                                     start=(c == 0), stop=(c == JC - 1))
                    for hh in range(2):
                        h = h0 + hh
                        if KN_PREM:
                            nc.tensor.matmul(g23[hh][:], lun_g[g][:, cc, h, :],
                                             m14[:, hh, :],
                                             start=False, stop=(c == JC - 1))
                        elif xor_chunk:
                            nc.tensor.matmul(g23[hh][:], lup_g[g][:, cc, h, :],
                                             m0[:, hh, :],
                                             start=(c == 0), stop=(c == JC - 1))
                        else:
                            nc.tensor.matmul(g23[hh][:], lun_g[g][:, cc, h, :],
                                             m14[:, hh, :],
                                             start=(c == 0), stop=False)
                            nc.tensor.matmul(g23[hh][:], lup_g[g][:, cc, h, :],
                                             m_tiles[c][:, 0, :],
                                             start=False, stop=(c == JC - 1))

                if ph == 0 and KN_SPLIT:
                    for (lu, lup, lun, wh4, ub, u02b, u, u02) in deferred:
                        nc.vector.tensor_tensor(lu[:, :, 2:4, 0:DK],
                                                wh4[:, :, 2:4, :],
                                                ub[:, :, 2:4, :], ALU.mult)
                        nc.vector.tensor_copy(lu[:, :, 2:4, DK], u[:, :, 2:4])
                        nc.vector.tensor_tensor(lup[:, :, 2:4, 0:DK],
                                                wh4[:, :, 2:4, :],
                                                u02b[:, :, 2:4, :], ALU.mult)
                        nc.vector.tensor_copy(lup[:, :, 2:4, DK], u02[:, :, 2:4])
                        nc.vector.tensor_scalar_mul(lun[:, :, 2:4, :],
                                                    lup[:, :, 2:4, :], -1.0)

                # ---------- finalize heads: transpose, combine, divide ----------
                for hh in range(2):
                    h = h0 + hh
                    with nc.named_scope(f"fin{h}"):
                        g1sb = finpool.tile([D1, SH], F32, tag="g1sb")
                        g23sb = finpool.tile([D1, SH], F32, tag="g23sb")
                        nc.scalar.copy(g1sb[:], g1[hh][:])
                        nc.scalar.copy(g23sb[:], g23[hh][:])
                        for icx in range(ICN):
                            tp = whps.tile([P, 2 * D1], F32, tag="whp", name="tp")
                            isl = slice(icx * P, (icx + 1) * P)
                            nc.tensor.transpose(tp[:, 0:D1], g1sb[:, isl],
                                                ident[:D1, :D1])
                            nc.tensor.transpose(tp[:, D1 : 2 * D1], g23sb[:, isl],
                                                ident[:D1, :D1])
                            q1 = finpool.tile([P, D1], F32, tag="q1")
                            q2 = finpool.tile([P, D1], F32, tag="q2")
                            nc.vector.tensor_scalar_mul(
                                q1[:], tp[:, 0:D1], v_all[:, icx, h : h + 1])
                            nc.vector.scalar_tensor_tensor(
                                q2[:], tp[:, D1 : 2 * D1],
                                v02_all[:, icx, h : h + 1], q1[:],
                                op0=ALU.mult, op1=ALU.add)
                            rz = finpool.tile([P, 1], F32, tag="rz")
                            nc.vector.reciprocal(rz[:], q2[:, DK : DK + 1])
                            nc.vector.tensor_scalar_mul(
                                oT[icx][:, h * DK : (h + 1) * DK],
                                q2[:, 0:DK], rz[:])

        # ---------- elu + output ----------
        with nc.named_scope("elu_out"):
            for icx in range(ICN):
                xm = finpool.tile([P, OUT_DIM], F32, tag="xm")
                ge = finpool.tile([P, OUT_DIM], F32, tag="ge")
                r2 = finpool.tile([P, OUT_DIM], F32, tag="r2")
                nc.vector.tensor_scalar_min(xm[:], oT[icx][:], 0.0)
                nc.scalar.activation(ge[:], xm[:], AF.Exp)
                nc.vector.scalar_tensor_tensor(r2[:], oT[icx][:], 0.0, ge[:],
                                               op0=ALU.max, op1=ALU.add)
                nc.vector.tensor_scalar_add(r2[:], r2[:], -1.0)
                nc.sync.dma_start(out_e[icx * P : (icx + 1) * P, :], r2[:])

    nc.compile()
    return nc


_NC_CACHE = None


def _get_nc():
    global _NC_CACHE
    if _NC_CACHE is None:
        _NC_CACHE = build()
    return _NC_CACHE


def make_in_maps(H, A, W, a_l, a_r):
    H = np.asarray(H, dtype=np.float32)
    A = np.asarray(A, dtype=np.float32)
    W = np.asarray(W, dtype=np.float32)
    a_l = np.asarray(a_l, dtype=np.float32)
    a_r = np.asarray(a_r, dtype=np.float32)

    mask = (A > 0).astype(np.float32)
    np.fill_diagonal(mask, 1.0)  # A_tilde = A + I > 0

    hT = np.ascontiguousarray(H.T).astype(ml_dtypes.bfloat16)
    # wl[k,h] = sum_d W[h*DK+d, k] * a_l[h, d]
    W4 = W.reshape(HEADS, DK, IN_DIM)
    wl = np.einsum("hdk,hd->kh", W4, a_l).astype(np.float32)
    wr = np.einsum("hdk,hd->kh", W4, a_r).astype(np.float32)
    wrhs = np.ascontiguousarray(np.concatenate([W.T, wl, wr], axis=1))
    ident = np.eye(P, dtype=np.float32)

    in_maps = []
    for c in range(NCORES):
        rows = slice(c * SH, (c + 1) * SH)
        mT = np.ascontiguousarray(mask[rows, :].T).astype(ml_dtypes.bfloat16)
        hTo = np.ascontiguousarray(H[rows, :].T)
        in_maps.append({"mT": mT, "hT": hT, "hTo": hTo, "wrhs": wrhs,
                        "ident": ident})
    return in_maps


def run(H, A, W, a_l, a_r, trace=False):
    nc = _get_nc()
    in_maps = make_in_maps(H, A, W, a_l, a_r)
    res = run_bass_kernel_spmd(nc, in_maps, core_ids=list(range(NCORES)),
                               trace=trace)
    out = np.concatenate([res.results[c]["out"] for c in range(NCORES)], axis=0)
    return out, res


def kernel(H, A, W, a_l, a_r):
    out, _ = run(H, A, W, a_l, a_r, trace=False)
    return out
